# revision 12
# baseline (speedup 1.0000x reference)
"""DiffNet GNN message-passing kernel for 8 TRN2 NeuronCores (Bass/Tile).

Algorithm (matches reference.py):
    for (W, b) in ((W0,b0),(W1,b1)):
        U = relu(concat([S @ U, U], 1) @ W + b)
    user_g = U + R @ V
    return user_g[batch_user], V[batch_pos_item], V[batch_neg_item]

Key restructurings (output-equivalent):
  * Backward slicing: layer-1 rows and R rows are only needed at the 8192
    batch slots; layer-0 rows only at cols referenced by layer-1 (+batch).
  * All SpMMs are gather (dma_gather, 256B rows) + two-stage matmul
    segment-sum: per 128-edge chunk a one-hot A1 [128,32] (built on DVE from
    per-edge slot bytes) maps edges->slots in PSUM; a second one-hot A2
    (slots->row-position) reduces slots to rows. All access patterns are
    static and identical across cores (SPMD); per-core structure lives in
    DRAM contents (indices / slot bytes).
  * S values are constant 1/32 -> folded into top half of W0/W1.
    R values are constant 1/50 -> folded into a scaled PSUM->SBUF copy.
  * Row-parallel sharding: core c owns users [c*12500,(c+1)*12500); layer-1
    partial aggregates are exchanged with a single ReduceScatter.

v2 performance restructuring (same math):
  * Gathers batched: one multi-packet dma_gather per (window-group, bucket)
    (~3-7k descriptors each) instead of 1024-idx single-packet calls; the
    GPSIMD SWDGE prep path stops being the serial bottleneck and the 16 SDMA
    engines stay fed.
  * Per-window l1/l2 slot-byte loads replaced by one bf16 preload per phase
    (resident in SBUF); per-group index loads on the sync engine (HWDGE).
  * Segment-sum matmuls run in bf16 (1 PE cycle/row vs 4 for fp32): gathered
    rows are cast f32->bf16 on DVE, one-hots built directly in bf16.
    Weight/concat/epilogue math stays fp32.
  * PSUM->SBUF region copies batched (one activation per window).
"""

import math
import os
import sys

sys.path.insert(0, "/opt/trn_rl_repo")

import numpy as np
import ml_dtypes

BF16 = ml_dtypes.bfloat16

# ---------------------------------------------------------------- constants
P = 128          # partitions / chunk size
D = 64           # embedding dim
SLOTS = 32       # stage-1 slots per chunk (max distinct rows per chunk)
REGION = 4       # chunks per stage-1 psum region (4*32 = 128 slots)
NEG = -1         # pad value for l1/l2 (matches nothing in iota)
IDXC = P // 16   # idx16 columns per chunk
GMAX_CH = 8      # max chunks per dma_gather call (single-packet: 64
                 # descriptors/engine; multi-packet breaks the tile
                 # framework's +16-per-DMA semaphore accounting)


class Cfg:
    def __init__(self, num_users=100000, num_items=50000, ncores=8,
                 bucket=32768, win0=192, winr=128, win_l1=128, win_ep=128,
                 s_pad=1536, g_l0=2, g_r=2, g_l1=12):
        self.num_users = num_users
        self.num_items = num_items
        self.ncores = ncores
        self.upc = num_users // ncores          # users per core
        self.bucket = bucket                    # int16 gather bucket rows
        self.win0 = win0                        # L0 stage-2 window rows
        self.winr = winr                        # R stage-2 window rows
        self.win_l1 = win_l1                    # L1 partial window rows
        self.win_ep = win_ep                    # epilogue window rows
        self.s_pad = s_pad                      # padded own-slots per core
        self.g_l0 = g_l0                        # windows per L0 gather group
        self.g_r = g_r
        self.g_l1 = g_l1
        assert num_users % ncores == 0
        assert s_pad % winr == 0 and s_pad % win_ep == 0
        assert win_ep % P == 0

    @property
    def nb_u(self):  # buckets in the (global) U table
        return math.ceil(self.num_users / self.bucket)

    @property
    def nb_v(self):
        return math.ceil(self.num_items / self.bucket)


FULL = Cfg()


# ---------------------------------------------------------------- host prep
class PhasePlan:
    """Static (SPMD-shared) layout of one gather+segment-sum phase.

    idx16 columns are laid out group-major: for each window-group g, for
    each bucket b, the chunks of windows in g (window-major). One
    dma_gather per (g, b) covers a contiguous idx range.
    """

    def __init__(self, win, n_dest, nb, gsize):
        self.win = win
        self.nw = n_dest // win
        self.nb = nb
        self.gsize = gsize
        self.chunks_wb = np.zeros((self.nw, nb), np.int64)  # real chunks
        # filled by finalize():
        self.cw = None        # [nw] total real chunks
        self.cw_pad = None    # [nw] padded to REGION multiple
        self.regions = None   # [nw]
        self.idx_off = None   # [nw, nb] col offsets into idx16 array
        self.l1_off = None
        self.l2_off = None
        self.groups = None    # list of window ranges
        self.g_of_w = None
        self.gstart = None    # [ngroups] idx col start
        self.gcols = None     # [ngroups] idx cols in group
        self.gbuck = None     # [ngroups][nb] (chunk_col_offset_in_group, n_chunks)

    def finalize(self):
        self.cw = self.chunks_wb.sum(1)
        assert (self.cw >= 1).all()
        self.cw_pad = ((self.cw + REGION - 1) // REGION) * REGION
        self.regions = self.cw_pad // REGION
        assert (self.regions <= 16).all(), self.regions.max()
        self.groups = [list(range(g0, min(g0 + self.gsize, self.nw)))
                       for g0 in range(0, self.nw, self.gsize)]
        self.g_of_w = np.zeros(self.nw, np.int64)
        for gi, grp in enumerate(self.groups):
            for w in grp:
                self.g_of_w[w] = gi
        self.idx_off = np.zeros((self.nw, self.nb), np.int64)
        self.gstart, self.gcols, self.gbuck = [], [], []
        off = 0
        for grp in self.groups:
            g0 = off
            per_b = []
            for b in range(self.nb):
                b0 = off
                for w in grp:
                    self.idx_off[w, b] = off
                    off += self.chunks_wb[w, b] * IDXC
                per_b.append(((b0 - g0) // IDXC, (off - b0) // IDXC))
            self.gstart.append(g0)
            self.gcols.append(off - g0)
            self.gbuck.append(per_b)
        self.idx_cols = max(off, 1)
        self.l1_off = np.concatenate([[0], np.cumsum(self.cw_pad)])
        self.l2_off = np.concatenate([[0], np.cumsum(self.regions)])


def _wrap_idx(idx_flat):
    """[n] int -> [128, n/16] int16 'wrapped in 16 partitions, replicated'."""
    n = idx_flat.shape[0]
    assert n % 16 == 0
    a = idx_flat.reshape(n // 16, 16).T.astype(np.int16)  # [16, n/16]
    return np.tile(a, (8, 1))                              # [128, n/16]


def _chunk_edges(dest, col):
    """Greedy chunking of row-sorted edges: <=128 edges, <=SLOTS distinct dests.
    Returns list of (dest_arr, col_arr) per chunk."""
    chunks = []
    n = dest.shape[0]
    i = 0
    while i < n:
        j = min(i + P, n)
        d = dest[i:j]
        new = np.empty(d.shape[0], np.bool_)
        new[0] = True
        new[1:] = d[1:] != d[:-1]
        ranks = np.cumsum(new) - 1
        if ranks[-1] >= SLOTS:  # too many distinct rows -> cut early
            j = i + int(np.argmax(ranks >= SLOTS))
            d = dest[i:j]
        chunks.append((d, col[i:j]))
        i = j
    return chunks


def build_phase(cfg, plan, edges_per_core, bucket_of, local_of):
    """edges_per_core: list of (dest_pos, col) arrays, dest_pos in [0, nw*win).
    Returns (idx16, l1b, l2) per-core arrays; fills plan.chunks_wb.
    l1b/l2 are bf16 (small ints, exact)."""
    nc_, win, nb = cfg.ncores, plan.win, plan.nb
    percore = []  # per core: {(w,b): [chunk...]}, chunk=(dest,col)
    for c in range(nc_):
        dest, col = edges_per_core[c]
        w_id = dest // win
        b_id = bucket_of(col)
        order = np.lexsort((dest, b_id, w_id))
        dest, col, w_id, b_id = dest[order], col[order], w_id[order], b_id[order]
        m = {}
        key = w_id * nb + b_id
        bounds = np.concatenate([[0], np.nonzero(np.diff(key))[0] + 1, [key.shape[0]]])
        for s, e in zip(bounds[:-1], bounds[1:]):
            if s == e:
                continue
            w, b = int(w_id[s]), int(b_id[s])
            m[(w, b)] = _chunk_edges(dest[s:e], col[s:e])
        percore.append(m)
        for (w, b), ch in m.items():
            plan.chunks_wb[w, b] = max(plan.chunks_wb[w, b], len(ch))
    for w in range(plan.nw):
        if plan.chunks_wb[w].sum() == 0:
            plan.chunks_wb[w, 0] = 1
    plan.finalize()

    idx16s, l1bs, l2s = [], [], []
    for c in range(nc_):
        m = percore[c]
        idx16 = np.zeros((P, plan.idx_cols), np.int16)
        l1b = np.full((P, int(plan.cw_pad.sum())), NEG, np.float32)
        l2 = np.full((P, int(plan.regions.sum())), NEG, np.float32)
        for w in range(plan.nw):
            k_in_w = 0  # chunk index within window (bucket-major, real only)
            for b in range(nb):
                n_ch = int(plan.chunks_wb[w, b])
                if n_ch == 0:
                    continue
                chunks = m.get((w, b), [])
                idx_flat = np.zeros(n_ch * P, np.int64)
                for ci in range(n_ch):
                    k = k_in_w + ci
                    if ci < len(chunks):
                        d, col = chunks[ci]
                        ne = d.shape[0]
                        idx_flat[ci * P: ci * P + ne] = local_of(col)
                        new = np.empty(ne, np.bool_)
                        new[0] = True
                        new[1:] = d[1:] != d[:-1]
                        ranks = np.cumsum(new) - 1
                        l1b[:ne, plan.l1_off[w] + k] = ranks
                        drep = d[new]  # distinct dests, order of appearance
                        for s_i, dd in enumerate(drep):
                            g = k * SLOTS + s_i            # window slot id
                            l2[g % P, plan.l2_off[w] + g // P] = dd - w * plan.win
                    # else: pad chunk (idx 0, l1 stays NEG)
                co = plan.idx_off[w, b]
                idx16[:, co: co + n_ch * IDXC] = _wrap_idx(idx_flat)
                k_in_w += n_ch
        idx16s.append(idx16)
        l1bs.append(l1b)
        l2s.append(l2.astype(BF16))
    return idx16s, l1bs, l2s


def host_prep(cfg, inputs):
    """Returns (plans, per-core input dicts, assembly metadata)."""
    U = np.asarray(inputs["U"], np.float32)
    V = np.asarray(inputs["V"], np.float32)
    W0 = np.asarray(inputs["W0"], np.float32)
    b0 = np.asarray(inputs["b0"], np.float32)
    W1 = np.asarray(inputs["W1"], np.float32)
    b1 = np.asarray(inputs["b1"], np.float32)
    S_row = np.asarray(inputs["S_row"], np.int64)
    S_col = np.asarray(inputs["S_col"], np.int64)
    S_val = np.asarray(inputs["S_val"], np.float32)
    R_row = np.asarray(inputs["R_row"], np.int64)
    R_col = np.asarray(inputs["R_col"], np.int64)
    R_val = np.asarray(inputs["R_val"], np.float32)
    bu_idx = np.asarray(inputs["batch_user"], np.int64)
    bp_idx = np.asarray(inputs["batch_pos_item"], np.int64)
    bn_idx = np.asarray(inputs["batch_neg_item"], np.int64)
    nc_ = cfg.ncores

    # constant sparse values (fold into weights / scales)
    s_val = float(S_val[0]); assert np.all(S_val == s_val)
    r_val = float(R_val[0]); assert np.all(R_val == r_val)

    W0s = W0.copy(); W0s[:D] *= s_val
    W1s = W1.copy(); W1s[:D] *= s_val

    # ---- slot ownership
    owner = bu_idx // cfg.upc
    slots_per_core = [np.nonzero(owner == c)[0] for c in range(nc_)]
    n_slots = np.array([s.shape[0] for s in slots_per_core])
    assert n_slots.max() <= cfg.s_pad, n_slots.max()

    s_order = np.argsort(S_row, kind="stable")
    S_row_s, S_col_s = S_row[s_order], S_col[s_order]
    row_start = np.searchsorted(S_row_s, np.arange(cfg.num_users))
    row_end = np.searchsorted(S_row_s, np.arange(cfg.num_users) + 1)

    r_order = np.argsort(R_row, kind="stable")
    R_row_s, R_col_s = R_row[r_order], R_col[r_order]
    rrow_start = np.searchsorted(R_row_s, np.arange(cfg.num_users))
    rrow_end = np.searchsorted(R_row_s, np.arange(cfg.num_users) + 1)

    def edges_of_rows(rows, starts, ends, cols):
        cnt = ends[rows] - starts[rows]
        rep = np.repeat(np.arange(rows.shape[0]), cnt)
        tot = int(cnt.sum())
        col = np.empty(tot, np.int64)
        if tot:
            idx = np.concatenate([np.arange(starts[r], ends[r]) for r in rows])
            col = cols[idx]
        return rep, col

    # ---- needed rows for U1 (layer-1 output of L0)
    distinct_bu = np.unique(bu_idx)
    _, l1_cols_all = edges_of_rows(distinct_bu, row_start, row_end, S_col_s)
    needed1 = np.union1d(np.unique(l1_cols_all), distinct_bu)

    rows1_per_core = [needed1[(needed1 >= c * cfg.upc) & (needed1 < (c + 1) * cfg.upc)]
                      for c in range(nc_)]
    n_rows1 = np.array([r.shape[0] for r in rows1_per_core])
    r0_max = int(math.ceil(n_rows1.max() / cfg.win0) * cfg.win0)
    u1_pos = np.full(cfg.num_users, -1, np.int64)
    for c in range(nc_):
        u1_pos[rows1_per_core[c]] = np.arange(n_rows1[c])

    # ---------------- L0 phase (aggT, windows over r0_max, buckets over U)
    plan0 = PhasePlan(cfg.win0, r0_max, cfg.nb_u, cfg.g_l0)
    l0_edges = []
    for c in range(nc_):
        rows = rows1_per_core[c]
        rep, col = edges_of_rows(rows, row_start, row_end, S_col_s)
        l0_edges.append((rep, col))
    l0_idx, l0_l1, l0_l2 = build_phase(
        cfg, plan0, l0_edges,
        bucket_of=lambda col: col // cfg.bucket,
        local_of=lambda col: col % cfg.bucket)

    # U rows for the concat half, pre-transposed: [64, r0_max]
    u_selT = []
    for c in range(nc_):
        sel = np.zeros((r0_max, D), np.float32)
        sel[:n_rows1[c]] = U[rows1_per_core[c]]
        u_selT.append(np.ascontiguousarray(sel.T))

    # ---------------- L1 partial phase (rowmajor, global padded slot axis)
    n_gslot = nc_ * cfg.s_pad
    plan1 = PhasePlan(cfg.win_l1, n_gslot, 1, cfg.g_l1)
    gslot_of_slot = np.full(bu_idx.shape[0], -1, np.int64)
    for c in range(nc_):
        gslot_of_slot[slots_per_core[c]] = c * cfg.s_pad + np.arange(n_slots[c])
    l1_edges = []
    for c in range(nc_):
        rep, col = edges_of_rows(bu_idx, row_start, row_end, S_col_s)
        m = (col >= c * cfg.upc) & (col < (c + 1) * cfg.upc)
        dest = gslot_of_slot[rep[m]]
        l1_edges.append((dest, col[m]))
    l1_idx, l1_l1, l1_l2 = build_phase(
        cfg, plan1, l1_edges,
        bucket_of=lambda col: np.zeros_like(col),
        local_of=lambda col: u1_pos[col])
    for c in range(nc_):
        assert (u1_pos[l1_edges[c][1]] >= 0).all()

    # ---------------- R phase (rowmajor, own slots, buckets over V)
    planr = PhasePlan(cfg.winr, cfg.s_pad, cfg.nb_v, cfg.g_r)
    r_edges = []
    for c in range(nc_):
        sl = slots_per_core[c]
        rep, col = edges_of_rows(bu_idx[sl], rrow_start, rrow_end, R_col_s)
        r_edges.append((rep, col))
    r_idx, r_l1, r_l2 = build_phase(
        cfg, planr, r_edges,
        bucket_of=lambda col: col // cfg.bucket,
        local_of=lambda col: col % cfg.bucket)

    # ---------------- concat gather (U1[batch_user] for own slots)
    u1b_idx = []
    for c in range(nc_):
        ids = np.zeros(cfg.s_pad, np.int64)
        ids[:n_slots[c]] = u1_pos[bu_idx[slots_per_core[c]]]
        assert (ids >= 0).all()
        u1b_idx.append(_wrap_idx(ids))

    # ---------------- bp / bn gathers (bucketed by V bucket)
    def item_gather(idx_all):
        per_core_ids, per_core_ord = [], []
        counts = np.zeros((nc_, cfg.nb_v), np.int64)
        for c in range(nc_):
            ids = idx_all[slots_per_core[c]]
            b = ids // cfg.bucket
            ordr = np.argsort(b, kind="stable")
            per_core_ids.append(ids[ordr])
            per_core_ord.append(ordr)
            for bb in range(cfg.nb_v):
                counts[c, bb] = int((b == bb).sum())
        nmax = [int(math.ceil(max(counts[c, b] for c in range(nc_)) / P) * P) or P
                for b in range(cfg.nb_v)]
        out_cols = sum(nmax)
        idx16, orders = [], []
        for c in range(nc_):
            flat = np.zeros(out_cols, np.int64)
            off = 0
            src = 0
            order_rows = []
            for b in range(cfg.nb_v):
                nb_c = int(counts[c, b])
                ids_b = per_core_ids[c][src:src + nb_c]
                flat[off:off + nb_c] = ids_b % cfg.bucket
                order_rows.append(per_core_ord[c][src:src + nb_c])
                src += nb_c
                off += nmax[b]
            idx16.append(_wrap_idx(flat))
            orders.append((np.concatenate(order_rows) if order_rows else
                           np.zeros(0, np.int64), counts[c]))
        return idx16, orders, nmax

    bp_i16, bp_ord, bp_nmax = item_gather(bp_idx)
    bn_i16, bn_ord, bn_nmax = item_gather(bn_idx)

    plans = dict(cfg=cfg, plan0=plan0, plan1=plan1, planr=planr,
                 r0_max=r0_max, bp_nmax=bp_nmax, bn_nmax=bn_nmax,
                 r_scale=r_val)
    meta = dict(slots_per_core=slots_per_core, n_slots=n_slots,
                bp_ord=bp_ord, bn_ord=bn_ord)

    iota8 = np.tile(np.arange(SLOTS, dtype=np.float32), (P, 1))
    iota_max = max(cfg.win0, cfg.winr, cfg.win_l1, cfg.win_ep)
    iota_win = np.tile(np.arange(iota_max, dtype=np.float32), (P, 1)).astype(BF16)
    ident = np.eye(P, dtype=np.float32)

    in_maps = []
    for c in range(nc_):
        in_maps.append(dict(
            u_tab=U, v_tab=V,
            w0s=W0s, w1s=W1s, b0=b0.reshape(D, 1), b1=b1.reshape(D, 1),
            u_selT=u_selT[c],
            l0_idx=l0_idx[c], l0_l1=l0_l1[c], l0_l2=l0_l2[c],
            l1_idx=l1_idx[c], l1_l1=l1_l1[c], l1_l2=l1_l2[c],
            r_idx=r_idx[c], r_l1=r_l1[c], r_l2=r_l2[c],
            u1b_idx=u1b_idx[c],
            bp_idx16=bp_i16[c], bn_idx16=bn_i16[c],
            iota8=iota8, iota_win=iota_win, ident=ident,
        ))
    return plans, in_maps, meta


# ---------------------------------------------------------------- builder
def build_nc(plans):
    import concourse.bass as bass
    import concourse.mybir as mybir
    import concourse.tile as tile
    from concourse import bacc

    cfg = plans["cfg"]
    plan0, plan1, planr = plans["plan0"], plans["plan1"], plans["planr"]
    r0_max = plans["r0_max"]
    f32 = mybir.dt.float32
    f32r = mybir.dt.float32r
    bf16 = mybir.dt.bfloat16
    i16 = mybir.dt.int16
    AF = mybir.ActivationFunctionType
    OP = mybir.AluOpType

    nc = bacc.Bacc("TRN2", target_bir_lowering=False, debug=False,
                   num_devices=cfg.ncores, num_swdge_queues=4)
    qrr = [0]  # round-robin SWDGE queue for gather calls

    def next_q():
        q = qrr[0]
        qrr[0] = (q + 1) % 4
        return q

    def din(name, shape, dt):
        return nc.dram_tensor(name, list(shape), dt, kind="ExternalInput")

    u_tab = din("u_tab", (cfg.num_users, D), f32)
    v_tab = din("v_tab", (cfg.num_items, D), f32)
    w0s = din("w0s", (2 * D, D), f32)
    w1s = din("w1s", (2 * D, D), f32)
    b0 = din("b0", (D, 1), f32)
    b1 = din("b1", (D, 1), f32)
    u_selT = din("u_selT", (D, r0_max), f32)
    l0_idx = din("l0_idx", (P, plan0.idx_cols), i16)
    l0_l1 = din("l0_l1", (P, int(plan0.cw_pad.sum())), f32)
    l0_l2 = din("l0_l2", (P, int(plan0.regions.sum())), bf16)
    l1_idxT = din("l1_idx", (P, plan1.idx_cols), i16)
    l1_l1 = din("l1_l1", (P, int(plan1.cw_pad.sum())), f32)
    l1_l2 = din("l1_l2", (P, int(plan1.regions.sum())), bf16)
    r_idx = din("r_idx", (P, planr.idx_cols), i16)
    r_l1 = din("r_l1", (P, int(planr.cw_pad.sum())), f32)
    r_l2 = din("r_l2", (P, int(planr.regions.sum())), bf16)
    u1b_idx = din("u1b_idx", (P, cfg.s_pad // 16), i16)
    bp_idx16 = din("bp_idx16", (P, sum(plans["bp_nmax"]) // 16), i16)
    bn_idx16 = din("bn_idx16", (P, sum(plans["bn_nmax"]) // 16), i16)
    iota8 = din("iota8", (P, SLOTS), f32)
    iota_max = max(cfg.win0, cfg.winr, cfg.win_l1, cfg.win_ep)
    iota_win = din("iota_win", (P, iota_max), bf16)
    ident = din("ident", (P, P), f32)

    bu_out = nc.dram_tensor("bu_out", [cfg.s_pad, D], f32, kind="ExternalOutput")
    bp_out = nc.dram_tensor("bp_out", [sum(plans["bp_nmax"]), D], f32,
                            kind="ExternalOutput")
    bn_out = nc.dram_tensor("bn_out", [sum(plans["bn_nmax"]), D], f32,
                            kind="ExternalOutput")

    with tile.TileContext(nc) as tc:
        import contextlib
        ctx = contextlib.ExitStack()
        with ctx:
            dram = ctx.enter_context(tc.tile_pool(name="dram", bufs=1, space="DRAM"))
            consts = ctx.enter_context(tc.tile_pool(name="consts", bufs=1))
            idxp = ctx.enter_context(tc.tile_pool(name="idx", bufs=3))
            gp = ctx.enter_context(tc.tile_pool(name="gath", bufs=3))
            a1p = ctx.enter_context(tc.tile_pool(name="a1", bufs=3))
            a2p = ctx.enter_context(tc.tile_pool(name="a2", bufs=3))
            regp = ctx.enter_context(tc.tile_pool(name="regs", bufs=3))
            catp = ctx.enter_context(tc.tile_pool(name="cat", bufs=3))
            outp = ctx.enter_context(tc.tile_pool(name="outs", bufs=3))
            keepp = ctx.enter_context(tc.tile_pool(name="keep", bufs=1))
            ps1 = ctx.enter_context(tc.tile_pool(name="ps1", bufs=2, space="PSUM"))
            ps2 = ctx.enter_context(tc.tile_pool(name="ps2", bufs=2, space="PSUM"))
            ps3 = ctx.enter_context(tc.tile_pool(name="ps3", bufs=2, space="PSUM"))

            # constants / per-phase side data resident in SBUF
            w0s_t = consts.tile([2 * D, D], f32, tag="w0")
            nc.sync.dma_start(w0s_t[:], w0s[:])
            w1s_t = consts.tile([2 * D, D], f32, tag="w1")
            nc.sync.dma_start(w1s_t[:], w1s[:])
            b0_t = consts.tile([D, 1], f32, tag="b0")
            nc.sync.dma_start(b0_t[:], b0[:])
            b1_t = consts.tile([D, 1], f32, tag="b1")
            nc.sync.dma_start(b1_t[:], b1[:])
            iota8f_t = consts.tile([P, SLOTS], f32, tag="io8")
            nc.sync.dma_start(iota8f_t[:], iota8[:])
            iota_win_t = consts.tile([P, iota_max], bf16, tag="iow")
            nc.sync.dma_start(iota_win_t[:], iota_win[:])
            ident_t = consts.tile([P, P], f32, tag="id")
            nc.sync.dma_start(ident_t[:], ident[:])
            zeros_t = consts.tile([P, D], f32, tag="z")
            nc.vector.memset(zeros_t[:], 0.0)

            l0_l1_t = consts.tile([P, int(plan0.cw_pad.sum())], f32, tag="l0l1")
            nc.sync.dma_start(l0_l1_t[:], l0_l1[:])
            l0_l2_t = consts.tile([P, int(plan0.regions.sum())], bf16, tag="l0l2")
            nc.sync.dma_start(l0_l2_t[:], l0_l2[:])
            l1_l1_t = consts.tile([P, int(plan1.cw_pad.sum())], f32, tag="l1l1")
            nc.sync.dma_start(l1_l1_t[:], l1_l1[:])
            l1_l2_t = consts.tile([P, int(plan1.regions.sum())], bf16, tag="l1l2")
            nc.sync.dma_start(l1_l2_t[:], l1_l2[:])
            r_l1_t = consts.tile([P, int(planr.cw_pad.sum())], f32, tag="rl1")
            nc.sync.dma_start(r_l1_t[:], r_l1[:])
            r_l2_t = consts.tile([P, int(planr.regions.sum())], bf16, tag="rl2")
            nc.sync.dma_start(r_l2_t[:], r_l2[:])

            u1_dram = dram.tile([r0_max, D], f32, tag="u1")
            partial_dram = dram.tile([cfg.ncores * cfg.s_pad, D], f32, tag="part")
            rs_out = dram.tile([cfg.s_pad, D], f32, tag="rsout")
            ragg_dram = dram.tile([cfg.s_pad, D], f32, tag="raggd")

            def table_slice(tab, n_rows, b):
                lo = b * cfg.bucket
                hi = min(lo + cfg.bucket, n_rows)
                return tab[lo:hi, :]

            def group_gather(plan, gi, idx_dram, tab, tab_rows):
                """Load the group's idx cols, issue one multi-packet gather
                per bucket into a single group tile. Returns the tile."""
                g0, gc = plan.gstart[gi], plan.gcols[gi]
                it = idxp.tile([P, gc], i16, tag="idx")
                nc.sync.dma_start(it[:], idx_dram[:, g0:g0 + gc])
                gt = gp.tile([P, gc // IDXC, D], f32, tag="g")
                for b in range(plan.nb):
                    c0, n_ch = plan.gbuck[gi][b]
                    # stay under the ~8k-descriptor per-call ceiling
                    for s0 in range(0, n_ch, GMAX_CH):
                        cc = min(GMAX_CH, n_ch - s0)
                        nc.gpsimd.dma_gather(
                            gt[:, c0 + s0:c0 + s0 + cc, :],
                            table_slice(tab, tab_rows, b),
                            it[:, (c0 + s0) * IDXC:(c0 + s0 + cc) * IDXC],
                            cc * P, cc * P, D, queue_num=next_q())
                return gt

            def window_stage12(plan, w, gi, gt, l1_t, l2_t, iota_t, win):
                """Build one-hots, run stage-1 matmuls straight off the
                gathered f32 rows (as float32r), return (a2, regs, nreg)."""
                g0 = plan.gstart[gi]
                cwp = int(plan.cw_pad[w])
                nreg = int(plan.regions[w])
                # per-chunk rhs slices (bucket-major chunk order == l1 bytes)
                rhss = []
                for b in range(plan.nb):
                    n_ch = int(plan.chunks_wb[w, b])
                    wc = (int(plan.idx_off[w, b]) - g0) // IDXC
                    for ci in range(n_ch):
                        rhss.append(gt[:, wc + ci, :])
                lo = int(plan.l1_off[w])
                a1 = a1p.tile([P, cwp, SLOTS], f32, tag="a1")
                nc.vector.tensor_tensor(
                    out=a1[:],
                    in0=l1_t[:, lo:lo + cwp].to_broadcast([P, cwp, SLOTS]),
                    in1=iota8f_t[:][:, None, :].to_broadcast([P, cwp, SLOTS]),
                    op=OP.is_equal)
                psum1 = ps1.tile([P, 16, D], f32, tag="ps1")
                for k in range(cwp):
                    p0 = SLOTS * (k % REGION)
                    rhs = rhss[k] if k < len(rhss) else zeros_t[:]
                    nc.tensor.matmul(
                        psum1[p0: p0 + SLOTS, k // REGION, :],
                        lhsT=a1[:, k, :], rhs=rhs,
                        start=True, stop=True, tile_position=(0, p0))
                regs = regp.tile([P, 16, D], bf16, tag="reg")
                nc.scalar.activation(regs[:, :nreg, :], psum1[:, :nreg, :],
                                     AF.Copy)
                lo2 = int(plan.l2_off[w])
                a2 = a2p.tile([P, nreg, win], bf16, tag="a2")
                nc.vector.tensor_tensor(
                    out=a2[:],
                    in0=l2_t[:, lo2:lo2 + nreg].to_broadcast([P, nreg, win]),
                    in1=iota_t[:, :win][:, None, :].to_broadcast([P, nreg, win]),
                    op=OP.is_equal)
                return a2, regs, nreg

            def transpose_out(srcT, win, dest_dram, row0):
                """srcT [64, win] sbuf f32 -> row-major [win, D] in dest_dram."""
                off = 0
                while off < win:
                    n = min(P, win - off)
                    pt = ps3.tile([P, D], f32, tag="tp")
                    nc.tensor.transpose(pt[:n, :], srcT[:, off:off + n],
                                        ident_t[:D, :D])
                    ot = outp.tile([P, D], f32, tag="o")
                    nc.scalar.activation(ot[:n, :], pt[:n, :], AF.Copy)
                    nc.scalar.dma_start(dest_dram[row0 + off:row0 + off + n, :],
                                        ot[:n, :])
                    off += n

            # ================= L0 =================
            for gi, grp in enumerate(plan0.groups):
                gt = group_gather(plan0, gi, l0_idx, u_tab, cfg.num_users)
                for w in grp:
                    a2, regs, nreg = window_stage12(
                        plan0, w, gi, gt, l0_l1_t, l0_l2_t,
                        iota_win_t, cfg.win0)
                    psum2 = ps2.tile([D, cfg.win0], f32, tag="ps2")
                    for r in range(nreg):
                        nc.tensor.matmul(psum2[:], lhsT=regs[:, r, :],
                                         rhs=a2[:, r, :],
                                         start=(r == 0), stop=(r == nreg - 1))
                    cat = catp.tile([2 * D, cfg.win0], f32, tag="cat")
                    nc.scalar.activation(cat[:D, :], psum2[:], AF.Copy)
                    nc.sync.dma_start(
                        cat[D:, :],
                        u_selT[:, w * cfg.win0:(w + 1) * cfg.win0])
                    psw = ps2.tile([D, cfg.win0], f32, tag="ps2")
                    nc.tensor.matmul(psw[:], lhsT=w0s_t[:], rhs=cat[:],
                                     start=True, stop=True)
                    u1T = outp.tile([D, cfg.win0], f32, tag="u1T")
                    nc.scalar.activation(u1T[:], psw[:], AF.Relu, bias=b0_t[:])
                    transpose_out(u1T, cfg.win0, u1_dram, w * cfg.win0)

            tc.strict_bb_all_engine_barrier()

            # ========= tail: L1 partials + R + bp/bn + u1b interleaved =========
            # L1 needs U1 (hence the barrier); R / bp / bn are independent and
            # fill the DMA idle time while L1 windows wait on dependencies.
            def emit_l1_group(gi):
                grp = plan1.groups[gi]
                gt = group_gather(plan1, gi, l1_idxT, u1_dram, r0_max)
                for w in grp:
                    a2, regs, nreg = window_stage12(
                        plan1, w, gi, gt, l1_l1_t, l1_l2_t,
                        iota_win_t, cfg.win_l1)
                    psum2 = ps2.tile([P, D], f32, tag="ps2")
                    for r in range(nreg):
                        nc.tensor.matmul(psum2[:], lhsT=a2[:, r, :],
                                         rhs=regs[:, r, :],
                                         start=(r == 0), stop=(r == nreg - 1))
                    po = outp.tile([P, D], f32, tag="po")
                    nc.scalar.activation(po[:], psum2[:], AF.Copy)
                    nc.scalar.dma_start(
                        partial_dram[w * cfg.win_l1:(w + 1) * cfg.win_l1, :],
                        po[:])

            def emit_r_group(gi):
                grp = planr.groups[gi]
                gt = group_gather(planr, gi, r_idx, v_tab, cfg.num_items)
                for w in grp:
                    a2, regs, nreg = window_stage12(
                        planr, w, gi, gt, r_l1_t, r_l2_t,
                        iota_win_t, cfg.winr)
                    psum2 = ps2.tile([P, D], f32, tag="ps2")
                    for r in range(nreg):
                        nc.tensor.matmul(psum2[:], lhsT=a2[:, r, :],
                                         rhs=regs[:, r, :],
                                         start=(r == 0), stop=(r == nreg - 1))
                    po = outp.tile([P, D], f32, tag="po")
                    nc.scalar.activation(po[:], psum2[:], AF.Copy)
                    nc.scalar.dma_start(
                        ragg_dram[w * cfg.winr:(w + 1) * cfg.winr, :], po[:])

            # u1b gather early (only needs u1_dram)
            u1b_g = keepp.tile([P, cfg.s_pad // P, D], f32, tag="u1b")
            itb = idxp.tile([P, cfg.s_pad // 16], i16, tag="idxu1b")
            nc.sync.dma_start(itb[:], u1b_idx[:])
            for s0 in range(0, cfg.s_pad // P, GMAX_CH):
                cc = min(GMAX_CH, cfg.s_pad // P - s0)
                nc.gpsimd.dma_gather(
                    u1b_g[:, s0:s0 + cc, :], u1_dram[:],
                    itb[:, s0 * IDXC:(s0 + cc) * IDXC],
                    cc * P, cc * P, D, queue_num=next_q())

            def emit_bpn(which):
                idx_t, nmaxs, outt = which
                it = idxp.tile([P, sum(nmaxs) // 16], i16, tag="idxb")
                nc.sync.dma_start(it[:], idx_t[:])
                gt = gp.tile([P, sum(nmaxs) // P, D], f32, tag="gb2")
                off = 0
                for b, nmax in enumerate(nmaxs):
                    for s0 in range(0, nmax // P, GMAX_CH):
                        cc = min(GMAX_CH, nmax // P - s0)
                        nc.gpsimd.dma_gather(
                            gt[:, off // P + s0:off // P + s0 + cc, :],
                            table_slice(v_tab, cfg.num_items, b),
                            it[:, off // 16 + s0 * IDXC:
                               off // 16 + (s0 + cc) * IDXC],
                            cc * P, cc * P, D, queue_num=next_q())
                    off += nmax
                nc.sync.dma_start(
                    outt[:].rearrange("(c p) e -> p c e", p=P), gt[:])

            bpn = [(bp_idx16, plans["bp_nmax"], bp_out),
                   (bn_idx16, plans["bn_nmax"], bn_out)]
            n1, nr = len(plan1.groups), len(planr.groups)
            ri = 0
            for gi in range(n1):
                emit_l1_group(gi)
                if ri < nr:
                    emit_r_group(ri)
                    ri += 1
                if gi in (1, 3) and bpn:
                    emit_bpn(bpn.pop(0))
            while ri < nr:
                emit_r_group(ri)
                ri += 1
            while bpn:
                emit_bpn(bpn.pop(0))

            tc.strict_bb_all_engine_barrier()
            nc.gpsimd.collective_compute(
                "ReduceScatter", OP.add,
                replica_groups=[list(range(cfg.ncores))],
                ins=[partial_dram.opt()], outs=[rs_out.opt()])
            tc.strict_bb_all_engine_barrier()

            # ================= L1 epilogue: own slots =================
            we = cfg.win_ep
            for w in range(cfg.s_pad // we):
                cat = catp.tile([2 * D, we], f32, tag="cat")
                rt = regp.tile([P, we // P, D], f32, tag="rt")
                nc.sync.dma_start(
                    rt[:], rs_out[w * we:(w + 1) * we, :]
                    .rearrange("(c p) e -> p c e", p=P))
                for j in range(we // P):
                    pt = ps3.tile([D, P], f32, tag="tp")
                    nc.tensor.transpose(pt[:], rt[:, j, :], ident_t[:])
                    nc.scalar.activation(cat[:D, j * P:(j + 1) * P], pt[:], AF.Copy)
                    pt2 = ps3.tile([D, P], f32, tag="tp")
                    nc.tensor.transpose(
                        pt2[:], u1b_g[:, (w * we) // P + j, :], ident_t[:])
                    nc.scalar.activation(cat[D:, j * P:(j + 1) * P], pt2[:], AF.Copy)
                psw = ps2.tile([D, we], f32, tag="ps2")
                nc.tensor.matmul(psw[:], lhsT=w1s_t[:], rhs=cat[:],
                                 start=True, stop=True)
                ugT = outp.tile([D, we], f32, tag="ugT")
                nc.scalar.activation(ugT[:], psw[:], AF.Relu, bias=b1_t[:])
                rt2 = regp.tile([P, we // P, D], f32, tag="rt")
                nc.sync.dma_start(
                    rt2[:], ragg_dram[w * we:(w + 1) * we, :]
                    .rearrange("(c p) e -> p c e", p=P))
                radd = catp.tile([D, we], f32, tag="radd")
                for j in range(we // P):
                    pt3 = ps3.tile([D, P], f32, tag="tp")
                    nc.tensor.transpose(pt3[:], rt2[:, j, :], ident_t[:])
                    nc.scalar.activation(radd[:, j * P:(j + 1) * P], pt3[:],
                                         AF.Copy, scale=plans["r_scale"])
                nc.vector.tensor_add(out=ugT[:], in0=ugT[:], in1=radd[:])
                transpose_out(ugT, we, bu_out, w * we)

    nc.compile()
    return nc


# ---------------------------------------------------------------- assembly
def assemble(plans, meta, results):
    cfg = plans["cfg"]
    B = sum(len(s) for s in meta["slots_per_core"])
    bu = np.zeros((B, D), np.float32)
    bp = np.zeros((B, D), np.float32)
    bn = np.zeros((B, D), np.float32)
    for c in range(cfg.ncores):
        sl = meta["slots_per_core"][c]
        n = len(sl)
        bu[sl] = results[c]["bu_out"][:n]
        for nm, arr, ords, nmaxs in (("bp_out", bp, meta["bp_ord"], plans["bp_nmax"]),
                                     ("bn_out", bn, meta["bn_ord"], plans["bn_nmax"])):
            rows = results[c][nm]
            order, counts = ords[c]
            src_rows = []
            off = 0
            for b, nmax in enumerate(nmaxs):
                src_rows.append(np.arange(off, off + counts[b]))
                off += nmax
            src_rows = np.concatenate(src_rows) if src_rows else np.zeros(0, np.int64)
            arr[sl[order]] = rows[src_rows]
    return bu, bp, bn


# ---------------------------------------------------------------- entry
def _install_ntff_shim():
    """antenv.axon_hooks is absent in some agent images; provide it and
    register the ctypes NTFF profiler so trace=True works under axon."""
    import types
    try:
        import antenv.axon_hooks  # noqa: F401
        return
    except ImportError:
        pass
    mod = types.ModuleType("antenv.axon_hooks")
    _hook = [None]
    mod.set_axon_ntff_profile_hook = lambda h: _hook.__setitem__(0, h)
    mod.get_axon_ntff_profile_hook = lambda: _hook[0]
    sys.modules["antenv.axon_hooks"] = mod
    import antenv
    antenv.axon_hooks = mod
    try:
        if "/root/.axon_site" not in sys.path:
            sys.path.append("/root/.axon_site")
        from trn_agent_boot.trn_boot import _ntff_profile_via_ctypes
        mod.set_axon_ntff_profile_hook(
            _ntff_profile_via_ctypes("/opt/axon/libaxon_pjrt.so"))
    except Exception:
        pass


def kernel(**inputs):
    cfg = FULL
    plans, in_maps, meta = host_prep(cfg, inputs)
    nc = build_nc(plans)
    trace = bool(int(os.environ.get("KERNEL_TRACE", "0")))
    if trace:
        _install_ntff_shim()
    from concourse.bass_utils import run_bass_kernel_spmd
    res = run_bass_kernel_spmd(nc, in_maps, list(range(cfg.ncores)),
                               trace=trace)
    out = assemble(plans, meta, res.results)
    kernel.last_exec_time_ns = res.exec_time_ns
    kernel.last_results = res
    return out


kernel.last_exec_time_ns = None
kernel.last_results = None
